# revision 6
# baseline (speedup 1.0000x reference)
"""Trainium2 Bass kernel for nn_CompressiveMemory_57750130262084.

The reference computes (B=8, S=4096, DK=DV=1024):
    sigma  = elu(query) + 1                                  [B,S,DK]
    memory = einsum('bkd,bsv->bkv', swap(sigma), value)      [B,DK,DV]
    z_norm = sum_s sigma                                     [B,DK]
    out    = einsum('bsd,bkv->bsv', sigma, memory)
           / einsum('bsd,bk->bs',  sigma, z_norm)[..., None]

Every einsum uses disjoint summed subscripts, so each factorises into
outer products of independent reductions; the algebra collapses to
    out[b,s,v] = sum_s value[b,s,v]        (exactly; query cancels)

So the kernel is a column-sum of `value` over S, broadcast over S.
Sharding: data-parallel over batch, one NeuronCore per batch element.
Per-core work: read 16 MB, reduce 4096 rows -> 1 row, write 16 MB.

v3 schedule (from trace analysis of v1 @ 120.3us and a failed v2):
  * SDMA engine 15 is ~12% slower per 4KB packet than engines 0-14
    (181ns vs 161ns, both directions). HWDGE deals a DMA's descriptor
    list to engines in contiguous blocks of ceil(N/16) starting at
    engine 0 (confirmed empirically: per-engine descriptor counts
    match exactly), so a DMA built over 127 partitions (N = 127*w
    descriptors) hands engine 15 exactly 7/8 of the other engines'
    load -- offsetting its slowness. All transfers here use
    partitions 0..126; partition 127 is unused. Rows are laid out in
    "slots" of 127: slot s covers DRAM rows [127s, 127s+127), row
    127s+p -> partition p. The 32 leftover rows [4064,4096) go to
    partitions 0..31 in one small DMA.
  * No per-chunk PE matmuls (fp32 matmul is 4 passes/chunk and
    backlogged v1's tail by ~13us). The DVE folds each input window
    into a per-window partial (tensor_add chain, ~1.23us/slot); the
    PE reduces each finished partial across partitions into
    accumulating PSUM while later windows stream in.
  * The last three windows (2+1+1 slots) go through bf16 (ACT
    convert) so their PE passes are single-pass; the post-stream tail
    is ~4us instead of ~17us. bf16 on ~12% of the data adds ~6e-4
    relative error (harness gate is 2e-2).
  * Output is broadcast-source DMAs (all output rows are identical)
    over the same 127-partition geometry, so the write stream gets
    the same engine-15 skew; the final write DMA is small so its
    completion receipt isn't behind megabytes of data.
"""

import numpy as np

B, S, D = 8, 4096, 1024
P = 128
PD = 127                # partitions actually used by DMAs (engine-15 skew)
NSLOT = 32              # 32 slots x 127 rows = 4064 rows
LEFT0 = NSLOT * PD      # rows [4064, 4096) -> partitions 0..31
NLEFT = S - LEFT0
H = 512                 # PSUM bank width in f32 (matmul N limit)

IN_WINDOWS = [(0, 5), (5, 10), (10, 15), (15, 20), (20, 25), (25, 28),
              (28, 30), (30, 31), (31, 32)]
BF16_WINDOWS = {6, 7, 8}
OUT_WINDOWS = [(0, 11), (11, 22), (22, 30), (30, 32)]

_CACHE: dict = {}


def _build_program():
    import concourse.mybir as mybir
    import concourse.tile as tile
    from concourse import bacc

    f32 = mybir.dt.float32
    bf16 = mybir.dt.bfloat16
    nc = bacc.Bacc("TRN2", target_bir_lowering=False, debug=False,
                   num_devices=B, enable_asserts=False)
    v = nc.declare_dram_parameter("value", [S, D], f32, isOutput=False)
    o = nc.declare_dram_parameter("out", [S, D], f32, isOutput=True)

    def in_slots(a, b):
        return v[PD * a : PD * b].rearrange("(n p) m -> p n m", p=PD)

    def out_slots(a, b):
        return o[PD * a : PD * b].rearrange("(n p) m -> p n m", p=PD)

    with tile.TileContext(nc) as tc:
        with (
            tc.tile_pool(name="in", bufs=1) as in_pool,
            tc.tile_pool(name="part", bufs=1) as part_pool,
            tc.tile_pool(name="ones", bufs=1) as ones_pool,
            tc.tile_pool(name="bcast", bufs=1) as bcast_pool,
            tc.tile_pool(name="psum", bufs=1, space="PSUM") as psum_pool,
        ):
            ones_f = ones_pool.tile([P, P], f32, tag="ones_f")
            nc.vector.memset(ones_f[:], 1.0)
            ones_b = ones_pool.tile([P, P], bf16, tag="ones_b")
            nc.vector.memset(ones_b[:], 1.0)

            ps = psum_pool.tile([P, D], f32)

            wtiles = []
            for wi, (a, b) in enumerate(IN_WINDOWS):
                wtiles.append(in_pool.tile([P, (b - a) * D], f32, tag=f"w{wi}", name=f"w{wi}"))
            lt = in_pool.tile([P, D], f32, tag="left")

            # Input DMAs, in window order; leftover rows early.
            for wi, (a, b) in enumerate(IN_WINDOWS):
                dst = wtiles[wi][0:PD].rearrange("p (n m) -> p n m", n=b - a)
                nc.sync.dma_start(dst, in_slots(a, b))
                if wi == 0:
                    nc.sync.dma_start(
                        lt[0:NLEFT].rearrange("p (n m) -> p n m", n=1),
                        v[LEFT0:S].rearrange("(n p) m -> p n m", p=NLEFT),
                    )

            # Fold + per-window partial reduction into PSUM.
            n_pe = 0
            total_pe = len(IN_WINDOWS)
            for wi, (a, b) in enumerate(IN_WINDOWS):
                t = wtiles[wi]
                n = b - a
                if n == 1:
                    partial = t
                else:
                    partial = part_pool.tile([P, D], f32, tag=f"p{wi % 3}")
                    nc.vector.tensor_add(partial[0:PD], t[0:PD, 0:D], t[0:PD, D : 2 * D])
                    for i in range(2, n):
                        nc.vector.tensor_add(partial[0:PD], partial[0:PD], t[0:PD, i * D : (i + 1) * D])
                if wi == 0:
                    nc.vector.tensor_add(partial[0:NLEFT], partial[0:NLEFT], lt[0:NLEFT])
                if wi in BF16_WINDOWS:
                    pb = part_pool.tile([P, D], bf16, tag=f"pb{wi % 3}")
                    nc.scalar.copy(pb[0:PD], partial[0:PD])
                    mm_in, mm_ones = pb, ones_b
                else:
                    mm_in, mm_ones = partial, ones_f
                for h in range(2):
                    nc.tensor.matmul(
                        ps[:, h * H : (h + 1) * H],
                        mm_ones[0:PD],
                        mm_in[0:PD, h * H : (h + 1) * H],
                        start=(n_pe == 0),
                        stop=(n_pe == total_pe - 1),
                    )
                n_pe += 1

            # PSUM -> SBUF in parallel halves (DVE + ACT).
            bc = bcast_pool.tile([P, D], f32)
            nc.vector.tensor_copy(bc[:, 0:H], ps[:, 0:H])
            nc.scalar.copy(bc[:, H:D], ps[:, H:D])

            # Output: broadcast bc rows to all DRAM rows, same geometry.
            for (a, b) in OUT_WINDOWS:
                src = bc[0:PD].unsqueeze(1).to_broadcast((PD, b - a, D))
                nc.sync.dma_start(out_slots(a, b), src)
            nc.sync.dma_start(
                o[LEFT0:S].rearrange("(n p) m -> p n m", p=NLEFT),
                bc[0:NLEFT].unsqueeze(1).to_broadcast((NLEFT, 1, D)),
            )

    nc.compile()
    return nc


def _get_program():
    if "nc" not in _CACHE:
        _CACHE["nc"] = _build_program()
    return _CACHE["nc"]


def kernel(query: np.ndarray, value: np.ndarray) -> np.ndarray:
    from concourse.bass_utils import run_bass_kernel_spmd

    del query  # output is exactly independent of query (see module docstring)
    value = np.ascontiguousarray(value, dtype=np.float32)
    assert value.shape == (B, S, D)

    nc = _get_program()
    in_maps = [{"value": value[b]} for b in range(B)]
    try:
        res = run_bass_kernel_spmd(nc, in_maps, list(range(B)))
    except Exception:
        # The tunneled runtime occasionally surfaces a transient
        # NRT_EXEC_UNIT_UNRECOVERABLE on the first dispatch; retry once.
        import time

        time.sleep(2.0)
        res = run_bass_kernel_spmd(nc, in_maps, list(range(B)))
    return np.stack([res.results[b]["out"] for b in range(B)], axis=0)


# revision 7
# speedup vs baseline: 11.2224x; 11.2224x over previous
"""Trainium2 Bass kernel for nn_CompressiveMemory_57750130262084.

The reference computes (B=8, S=4096, DK=DV=1024):
    sigma  = elu(query) + 1                                  [B,S,DK]
    memory = einsum('bkd,bsv->bkv', swap(sigma), value)      [B,DK,DV]
    z_norm = sum_s sigma                                     [B,DK]
    out    = einsum('bsd,bkv->bsv', sigma, memory)
           / einsum('bsd,bk->bs',  sigma, z_norm)[..., None]

Every einsum uses disjoint summed subscripts, so each factorises into
outer products of independent reductions; the algebra collapses to
    out[b,s,v] = sum_s value[b,s,v]        (exactly; query cancels)

So the kernel is a column-sum of `value` over S, broadcast over S.
Sharding: data-parallel over batch, one NeuronCore per batch element.
Per-core work: read 16 MB, reduce 4096 rows -> 1 row, write 16 MB.

v4 schedule (from v1 @120.3us trace + probe experiments):
  * SDMA engine 15 is ~12% slower per 4KB packet than engines 0-14
    (181 vs 161ns, both directions); with a uniform layout it
    straggles ~6-8us at the end of BOTH the read and write streams.
    HWDGE descriptor dealing (probed empirically): block =
    ceil(partitions/16); partition-blocks go to engines 0..15 in
    order IF partitions %% block == 0, else the whole DMA lands on
    engine 0. A 120-partition DMA therefore puts NOTHING on engine
    15. Mix: 28 slots x 128 rows (uniform) + 480 rows via
    120-partition DMAs + 32 rows on partitions 0-31 => engine 15
    ~40.9us busy vs ~41.5us for engines 0-14. Row->partition
    assignment is free (everything is summed / output rows are all
    identical). APs keep an interleave dim between partition and
    column so the AP optimizer cannot merge a contiguous [p][m] into
    one giant descriptor (that collapse costs 10x).
  * No per-chunk PE matmuls (fp32 matmul backlogged v1's tail by
    ~13us). The DVE folds each input window into a per-window partial
    (tensor_add chain, ~1.23us/chunk); the PE reduces each finished
    partial across partitions into accumulating PSUM while later
    windows stream.
  * The two final (120-partition) windows go through bf16 (ACT
    convert, single-pass PE) so the post-stream tail is ~4.5us.
    bf16 on ~12% of the data adds ~6e-4 rel error (gate is 2e-2).
  * Output: broadcast-source DMAs, same engine-15 skew; the last
    write DMAs are small so the final completion receipt is quick.
"""

import numpy as np

B, S, D = 8, 4096, 1024
P = 128
H = 512                  # PSUM bank width in f32 (matmul N limit)
NA = 28                  # 128-row interleaved chunks (rows [0, 3584))
B0 = NA * P              # 3584: start of the 120-partition region
PB = 120
NB = 4                   # 4 slots x 120 rows (rows [3584, 4064))
L0 = B0 + PB * NB        # 4064: leftover rows -> partitions 0..31
NL = S - L0              # 32

A_WINDOWS = [(0, 5), (5, 10), (10, 15), (15, 20), (20, 25), (25, 28)]
B_WINDOWS = [(0, 2), (2, 4)]         # bf16 tail windows
OUT_REP = 7              # chunks per A output DMA (4 x 7 = 28)

_CACHE: dict = {}


def _build_program():
    import concourse.mybir as mybir
    import concourse.tile as tile
    from concourse import bacc

    f32 = mybir.dt.float32
    bf16 = mybir.dt.bfloat16
    nc = bacc.Bacc("TRN2", target_bir_lowering=False, debug=False,
                   num_devices=B, enable_asserts=False)
    v = nc.declare_dram_parameter("value", [S, D], f32, isOutput=False)
    o = nc.declare_dram_parameter("out", [S, D], f32, isOutput=True)

    v_rows = v[0:B0].rearrange("(c p) m -> c p m", p=P)            # [28][128][1024]
    vb = v[B0:L0].rearrange("(n p) m -> p n m", p=PB)              # [120][4][1024]
    o_rows = o[0:B0].rearrange("(i n p) m -> i p n m", i=NA // OUT_REP, n=OUT_REP, p=P)
    ob = o[B0:L0].rearrange("(n p) m -> p n m", p=PB)

    with tile.TileContext(nc) as tc:
        with (
            tc.tile_pool(name="in", bufs=1) as in_pool,
            tc.tile_pool(name="part", bufs=1) as part_pool,
            tc.tile_pool(name="ones", bufs=1) as ones_pool,
            tc.tile_pool(name="bcast", bufs=1) as bcast_pool,
            tc.tile_pool(name="psum", bufs=1, space="PSUM") as psum_pool,
        ):
            ones_f = ones_pool.tile([P, P], f32, tag="ones_f")
            nc.vector.memset(ones_f[:], 1.0)
            ones_b = ones_pool.tile([P, P], bf16, tag="ones_b")
            nc.vector.memset(ones_b[:], 1.0)

            ps = psum_pool.tile([P, D], f32)

            atiles = [
                in_pool.tile([P, (b - a) * D], f32, tag=f"wa{wi}", name=f"wa{wi}")
                for wi, (a, b) in enumerate(A_WINDOWS)
            ]
            btiles = [
                in_pool.tile([P, (b - a) * D], f32, tag=f"wb{wi}", name=f"wb{wi}")
                for wi, (a, b) in enumerate(B_WINDOWS)
            ]
            lt = in_pool.tile([P, D], f32, tag="left")

            # Input DMAs. A-windows: interleaved chunks over all 128
            # partitions (uniform engine deal). Leftover early, in two
            # column halves (a full [32][1024] would AP-merge into one
            # descriptor). B-windows last: 120-partition DMAs skip
            # engine 15 entirely.
            for wi, (a, b) in enumerate(A_WINDOWS):
                dst = atiles[wi][:].rearrange("p (n m) -> p n m", n=b - a)
                nc.sync.dma_start(dst, v_rows[a:b].rearrange("n p m -> p n m"))
                if wi == 0:
                    nc.sync.dma_start(lt[0:NL, 0:H].unsqueeze(1), v[L0:S, 0:H].unsqueeze(1))
                    nc.sync.dma_start(lt[0:NL, H:D].unsqueeze(1), v[L0:S, H:D].unsqueeze(1))
            for wi, (a, b) in enumerate(B_WINDOWS):
                dst = btiles[wi][0:PB].rearrange("p (n m) -> p n m", n=b - a)
                nc.sync.dma_start(dst, vb[:, a:b])

            # Folds + per-window partition-reduce into PSUM.
            mm = []  # (moving AP, ones AP, valid partitions)
            for wi, (a, b) in enumerate(A_WINDOWS):
                t = atiles[wi]
                n = b - a
                partial = part_pool.tile([P, D], f32, tag=f"p{wi % 3}", name=f"p{wi % 3}")
                nc.vector.tensor_add(partial[:], t[:, 0:D], t[:, D : 2 * D])
                for i in range(2, n):
                    nc.vector.tensor_add(partial[:], partial[:], t[:, i * D : (i + 1) * D])
                if wi == len(A_WINDOWS) - 1:
                    nc.vector.tensor_add(partial[0:NL], partial[0:NL], lt[0:NL])
                mm.append((partial, ones_f, P))
            for wi, (a, b) in enumerate(B_WINDOWS):
                t = btiles[wi]
                partial = part_pool.tile([P, D], f32, tag=f"pb_f{wi}", name=f"pb_f{wi}")
                nc.vector.tensor_add(partial[0:PB], t[0:PB, 0:D], t[0:PB, D : 2 * D])
                pb = part_pool.tile([P, D], bf16, tag=f"pb{wi}", name=f"pb{wi}")
                nc.scalar.copy(pb[0:PB], partial[0:PB])
                mm.append((pb, ones_b, PB))

            for k, (m_in, m_ones, np_) in enumerate(mm):
                for h in range(2):
                    nc.tensor.matmul(
                        ps[:, h * H : (h + 1) * H],
                        m_ones[0:np_],
                        m_in[0:np_, h * H : (h + 1) * H],
                        start=(k == 0),
                        stop=(k == len(mm) - 1),
                    )

            # PSUM -> SBUF in parallel halves (DVE + ACT).
            bc = bcast_pool.tile([P, D], f32)
            nc.vector.tensor_copy(bc[:, 0:H], ps[:, 0:H])
            nc.scalar.copy(bc[:, H:D], ps[:, H:D])

            # Output: broadcast bc to all rows with the same skew.
            for i in range(NA // OUT_REP):
                src = bc[:].unsqueeze(1).to_broadcast((P, OUT_REP, D))
                nc.sync.dma_start(o_rows[i], src)
            nc.sync.dma_start(ob, bc[0:PB].unsqueeze(1).to_broadcast((PB, NB, D)))
            nc.sync.dma_start(o[L0:S, 0:H].unsqueeze(1), bc[0:NL, 0:H].unsqueeze(1))
            nc.sync.dma_start(o[L0:S, H:D].unsqueeze(1), bc[0:NL, H:D].unsqueeze(1))

    nc.compile()
    return nc


def _get_program():
    if "nc" not in _CACHE:
        _CACHE["nc"] = _build_program()
    return _CACHE["nc"]


def kernel(query: np.ndarray, value: np.ndarray) -> np.ndarray:
    from concourse.bass_utils import run_bass_kernel_spmd

    del query  # output is exactly independent of query (see module docstring)
    value = np.ascontiguousarray(value, dtype=np.float32)
    assert value.shape == (B, S, D)

    nc = _get_program()
    in_maps = [{"value": value[b]} for b in range(B)]
    try:
        res = run_bass_kernel_spmd(nc, in_maps, list(range(B)))
    except Exception:
        # The tunneled runtime occasionally surfaces a transient
        # NRT_EXEC_UNIT_UNRECOVERABLE on the first dispatch; retry once.
        import time

        time.sleep(2.0)
        res = run_bass_kernel_spmd(nc, in_maps, list(range(B)))
    return np.stack([res.results[b]["out"] for b in range(B)], axis=0)
